# revision 9
# baseline (speedup 1.0000x reference)
"""DeepSetPred Trainium2 kernel: 3-layer token encoder MLP + segment-sum +
predictor MLP on 8 NeuronCores, ZERO collectives.

Key structural wins over the naive mapping:

1. The third encoder layer is linear, so it commutes with the segment-sum:
       enc = segsum(h2 @ W3 + b3) = segsum(h2) @ W3 + counts * b3
   The per-token L3 matmul (25% of encoder FLOPs) and the one-hot
   segment-matmul collapse into one tiny [S,H]x[H,C] matmul after pooling.

2. segsum(h2) is computed without materializing anything token-major:
   - The L2 tanh on the ScalarEngine emits accum_out = sum over the whole
     chunk's tokens per H-partition (free side output).
   - Tokens are sorted by segment, so a 1024-token chunk contains at most
     two segment boundaries.  Two DVE scalar_tensor_tensor ops per
     (h-tile, chunk) compute head sums  sum_{t < b} h2[:, t]  via the fused
     (iota is_lt b) mult h2 -> accum_out form, with b per chunk as DATA
     (uniform instruction stream across all 8 cores -> one NEFF).
   - hsegT[h, s] = sum_c head1*(M1-M2) + head2*(M2-M3) + full*M3, with the
     combination matrices A/B/C as per-core input data, evaluated by tiny
     PE matmuls after transposing the [128, nchunk] accumulators.

3. Everything stays fp16 on the matmul path (fp8 fails: the ragged pooling
   amplifies per-token quantization noise by sqrt(count) ~ 32x).

Sharding: host cuts the sorted token axis at segment boundaries so every
segment belongs to exactly one core (no collectives); each core runs the
predictor on its own <=SLOTS segments and writes its slice of the output.
"""

import numpy as np

import concourse.mybir as mybir
import concourse.tile as tile
from concourse import bacc
from concourse import bass_utils
from concourse.masks import make_identity

# Problem shapes (hardcoded per contract).
T, E, H, C, O = 131072, 256, 512, 256, 32
S = 128            # num segments
N_CORES = 8
TOK = 1024         # tokens per chunk
NCPAD = 32         # padded chunk-count (accumulator columns)
SPLIT = 12         # accumulator columns assembled early (round 1)
MIN_SLOTS = 32     # baseline segments-per-core capacity
F32 = mybir.dt.float32
F32R = mybir.dt.float32r
F16 = mybir.dt.float16

_CACHE = {}


def _mm(nc, out, lhsT, rhs, start, stop, skip=True):
    nc.tensor.matmul(out, lhsT, rhs,
                     start=start, stop=stop, skip_group_check=skip)


def _build_nc(t_sh, SLOTS):
    assert t_sh % 128 == 0
    NC = (t_sh + TOK - 1) // TOK
    assert NC <= NCPAD

    nc = bacc.Bacc("TRN2", target_bir_lowering=False, debug=False,
                   num_devices=N_CORES)

    xt_d = nc.dram_tensor("xt", [E, t_sh], F16, kind="ExternalInput")
    w1_d = nc.dram_tensor("w1", [E, H], F16, kind="ExternalInput")
    w2_d = nc.dram_tensor("w2", [H, H], F16, kind="ExternalInput")
    w3_d = nc.dram_tensor("w3", [H, C], F32R, kind="ExternalInput")
    b1_d = nc.dram_tensor("b1", [H // 128, 128], F32, kind="ExternalInput")
    b2_d = nc.dram_tensor("b2", [H // 128, 128], F32, kind="ExternalInput")
    b3_d = nc.dram_tensor("b3", [1, C], F32, kind="ExternalInput")
    p1_d = nc.dram_tensor("p1", [C, H], F32R, kind="ExternalInput")
    p2_d = nc.dram_tensor("p2", [H, H], F32R, kind="ExternalInput")
    p3_d = nc.dram_tensor("p3", [H, O], F32R, kind="ExternalInput")
    pb1_d = nc.dram_tensor("pb1", [H // 128, 128], F32, kind="ExternalInput")
    pb2_d = nc.dram_tensor("pb2", [H // 128, 128], F32, kind="ExternalInput")
    pb3_d = nc.dram_tensor("pb3", [1, O], F32, kind="ExternalInput")
    cnt_d = nc.dram_tensor("cnt", [1, SLOTS], F32, kind="ExternalInput")
    iota_d = nc.dram_tensor("iota", [128, TOK], F16, kind="ExternalInput")
    bm1_d = nc.dram_tensor("bm1", [128, NCPAD], F32, kind="ExternalInput")
    bm2_d = nc.dram_tensor("bm2", [128, NCPAD], F32, kind="ExternalInput")
    cf1_d = nc.dram_tensor("cf1", [3 * SPLIT, SLOTS], F32,
                           kind="ExternalInput")
    cf2_d = nc.dram_tensor("cf2", [3 * (NCPAD - SPLIT), SLOTS], F32,
                           kind="ExternalInput")
    out_d = nc.dram_tensor("pred", [SLOTS, O], F32, kind="ExternalOutput")

    EC = E // 128   # 2
    HC = H // 128   # 4
    CC = C // 128   # 2
    LT = mybir.AluOpType.is_lt
    MUL = mybir.AluOpType.mult

    with tile.TileContext(nc) as tc:
        with tc.tile_pool(name="wts", bufs=1) as wp, \
             tc.tile_pool(name="xt", bufs=3) as xtp, \
             tc.tile_pool(name="h1", bufs=2) as h1p, \
             tc.tile_pool(name="h2", bufs=2) as h2p, \
             tc.tile_pool(name="scr", bufs=2) as scp, \
             tc.tile_pool(name="small", bufs=1) as smp, \
             tc.tile_pool(name="ps1", bufs=2, space="PSUM") as ps1p, \
             tc.tile_pool(name="ps2", bufs=2, space="PSUM") as ps2p:

            # warm the ACT tanh table before the scalar queue fills with DMAs
            warm_sb = smp.tile([1, 1], F32, tag="warm", name="warm")
            nc.gpsimd.memset(warm_sb[:], 0.0)
            warm_o = smp.tile([1, 1], F32, tag="warmo", name="warmo")
            nc.scalar.activation(warm_o[:], warm_sb[:],
                                 mybir.ActivationFunctionType.Tanh)

            # ---- resident weights ----
            w1_t = wp.tile([128, EC, HC, 128], F16, tag="w1", name="w1t")
            nc.scalar.dma_start(
                w1_t[:], w1_d.ap().rearrange("(e p) (h q) -> p e h q",
                                             p=128, q=128))
            w1_sb = [[w1_t[:, e, h, :] for h in range(HC)] for e in range(EC)]
            w2_t = wp.tile([128, HC, HC, 128], F16, tag="w2", name="w2t")
            nc.scalar.dma_start(
                w2_t[:], w2_d.ap().rearrange("(k p) (h q) -> p k h q",
                                             p=128, q=128))
            w2_sb = [[w2_t[:, k, h, :] for h in range(HC)] for k in range(HC)]
            w3_t = wp.tile([128, HC, C], F32R, tag="w3", name="w3t")
            nc.gpsimd.dma_start(
                w3_t[:], w3_d.ap().rearrange("(k p) c -> p k c", p=128))
            w3_sb = [w3_t[:, k, :] for k in range(HC)]
            # biases / rows (b1/b2 on the scalar ring: the L1/L2 tanh
            # needs them early and must not queue behind the big
            # epilogue-weight DMAs on the gpsimd ring)
            b1_sb = smp.tile([128, HC], F32, tag="b1", name="b1")
            nc.scalar.dma_start(b1_sb[:], b1_d.ap().rearrange("h p -> p h"))
            b2_sb = smp.tile([128, HC], F32, tag="b2", name="b2")
            nc.scalar.dma_start(b2_sb[:], b2_d.ap().rearrange("h p -> p h"))
            pb1_sb = smp.tile([128, HC], F32, tag="pb1", name="pb1")
            nc.gpsimd.dma_start(pb1_sb[:], pb1_d.ap().rearrange("h p -> p h"))
            pb2_sb = smp.tile([128, HC], F32, tag="pb2", name="pb2")
            nc.gpsimd.dma_start(pb2_sb[:], pb2_d.ap().rearrange("h p -> p h"))
            b3row = smp.tile([1, C], F32, tag="b3row", name="b3row")
            nc.gpsimd.dma_start(b3row[:], b3_d.ap())
            pb3row = smp.tile([1, O], F32, tag="pb3row", name="pb3row")
            nc.gpsimd.dma_start(pb3row[:], pb3_d.ap())
            cntrow = smp.tile([1, SLOTS], F32, tag="cntrow", name="cntrow")
            nc.gpsimd.dma_start(cntrow[:], cnt_d.ap())
            ones1 = smp.tile([1, SLOTS], F32, tag="ones1", name="ones1")
            nc.gpsimd.memset(ones1[:], 1.0)
            ident = smp.tile([128, 128], F32, tag="ident", name="ident")
            make_identity(nc, ident[:])
            iota_sb = smp.tile([128, TOK], F16, tag="iota", name="iota")
            nc.gpsimd.dma_start(iota_sb[:], iota_d.ap())
            bm1_sb = smp.tile([128, NCPAD], F32, tag="bm1", name="bm1")
            nc.gpsimd.dma_start(bm1_sb[:], bm1_d.ap())
            bm2_sb = smp.tile([128, NCPAD], F32, tag="bm2", name="bm2")
            nc.gpsimd.dma_start(bm2_sb[:], bm2_d.ap())
            cf1_sb = smp.tile([3 * SPLIT, SLOTS], F32, tag="cf1", name="cf1")
            nc.gpsimd.dma_start(cf1_sb[:], cf1_d.ap())
            cf2_sb = smp.tile([3 * (NCPAD - SPLIT), SLOTS], F32, tag="cf2",
                              name="cf2")
            nc.gpsimd.dma_start(cf2_sb[:], cf2_d.ap())

            p1_t = wp.tile([128, CC, HC, 128], F32R, tag="p1", name="p1t")
            nc.gpsimd.dma_start(
                p1_t[:], p1_d.ap().rearrange("(c p) (h q) -> p c h q",
                                             p=128, q=128))
            p1_sb = [[p1_t[:, c, h, :] for h in range(HC)] for c in range(CC)]
            p2_t = wp.tile([128, HC, HC, 128], F32R, tag="p2", name="p2t")
            nc.gpsimd.dma_start(
                p2_t[:], p2_d.ap().rearrange("(k p) (h q) -> p k h q",
                                             p=128, q=128))
            p2_sb = [[p2_t[:, k, h, :] for h in range(HC)] for k in range(HC)]
            p3_t = wp.tile([128, HC, O], F32R, tag="p3", name="p3t")
            nc.gpsimd.dma_start(
                p3_t[:], p3_d.ap().rearrange("(k p) o -> p k o", p=128))
            p3_sb = [p3_t[:, k, :] for k in range(HC)]

            # ---- per-(h,chunk) pooling accumulators (fp32), laid out
            # [h, chunk, plane] so a chunk-range slice is contiguous:
            # plane 0 = head1 sums, plane 1 = head2 sums, plane 2 = full ----
            acc_all = smp.tile([128, HC, NCPAD, 3], F32, tag="acc",
                               name="acc")
            nc.gpsimd.memset(acc_all[:], 0.0)

            # ---- main token loop ----
            chunks = []
            base = 0
            while base < t_sh:
                ct = min(TOK, t_sh - base)
                chunks.append((base, ct))
                base += ct

            def halves(ct):
                if ct <= 512:
                    return [(0, ct)]
                return [(0, 512), (512, ct - 512)]

            def load_and_l1(ci, base, ct):
                xt_t = xtp.tile([128, EC, ct], F16, tag="xt", name="xt",
                                padded_shape=[128, EC, TOK])
                nc.sync.dma_start(
                    xt_t[:],
                    xt_d.ap()[:, base:base + ct]
                        .rearrange("(e p) t -> p e t", p=128))
                h1_t = h1p.tile([128, HC, ct], F16, tag="h1", name="h1",
                                padded_shape=[128, HC, TOK])
                for h in range(HC):
                    ps1 = ps1p.tile([128, ct], F32, tag="ps1", name="ps1",
                                    padded_shape=[128, TOK])
                    for (hb, hl) in halves(ct):
                        for e in range(EC):
                            _mm(nc, ps1[:, hb:hb + hl], w1_sb[e][h],
                                xt_t[:, e, hb:hb + hl],
                                start=(e == 0), stop=(e == EC - 1))
                    nc.scalar.activation(h1_t[:, h, :], ps1[:],
                                         mybir.ActivationFunctionType.Tanh,
                                         bias=b1_sb[:, 0 + h:h + 1])
                return h1_t

            def l2_and_seg(ci, base, ct, h1_t):
                h2_t = h2p.tile([128, HC, ct], F16, tag="h2", name="h2",
                                padded_shape=[128, HC, TOK])
                for h in range(HC):
                    ps2 = ps2p.tile([128, ct], F32, tag="ps2", name="ps2",
                                    padded_shape=[128, TOK])
                    for (hb, hl) in halves(ct):
                        for k in range(HC):
                            _mm(nc, ps2[:, hb:hb + hl], w2_sb[k][h],
                                h1_t[:, k, hb:hb + hl],
                                start=(k == 0), stop=(k == HC - 1))
                    nc.scalar.activation(h2_t[:, h, :], ps2[:],
                                         mybir.ActivationFunctionType.Tanh,
                                         bias=b2_sb[:, h:h + 1],
                                         accum_out=acc_all[:, h, ci, 2:3])
                    # head sums before the (<=2) intra-chunk boundaries:
                    # (iota < b) * h2 summed along tokens, b is per-core data
                    for j, bm in enumerate((bm1_sb, bm2_sb)):
                        sc = scp.tile([128, ct], F16, tag=f"scr{j}",
                                      name=f"scr{j}", bufs=3,
                                      padded_shape=[128, TOK])
                        nc.vector.scalar_tensor_tensor(
                            sc[:], iota_sb[:, :ct], bm[:, ci:ci + 1],
                            h2_t[:, h, :], LT, MUL,
                            accum_out=acc_all[:, h, ci, j:j + 1])

            # round-1 accumulator assembly: transpose chunk columns
            # [0:SPLIT] once chunks 0..SPLIT-1 are done; overlaps the
            # remaining token chunks
            accT1 = smp.tile([3 * SPLIT, HC, 128], F32, tag="accT1",
                             name="accT1")

            def round1():
                for h in range(HC):
                    tp = ps1p.tile([3 * SPLIT, 128], F32, tag="ps1",
                                   name="tp1")
                    nc.tensor.transpose(tp[:], acc_all[:, h, 0:SPLIT, :],
                                        ident[:])
                    nc.vector.tensor_copy(accT1[:, h, :], tp[:])

            # software pipeline: L1(i+1) is emitted before L2/seg(i)
            pend = []
            done = 0
            for ci, (base, ct) in enumerate(chunks):
                pend.append((ci, base, ct, load_and_l1(ci, base, ct)))
                keep = 2 if ci < 2 else 1
                while len(pend) > keep:
                    l2_and_seg(*pend.pop(0))
                    done += 1
                    if done == SPLIT:
                        round1()
            while pend:
                l2_and_seg(*pend.pop(0))
                done += 1
                if done == SPLIT:
                    round1()

            # ---- assemble hsegT: round 2 over the remaining chunk
            # columns, then 2 tiny matmuls per h against the host-built
            # coefficient stacks (round 1 ran mid-loop, overlapped) ----
            accT2 = smp.tile([3 * (NCPAD - SPLIT), HC, 128], F32,
                             tag="accT2", name="accT2")
            for h in range(HC):
                tp = ps1p.tile([3 * (NCPAD - SPLIT), 128], F32, tag="ps1",
                               name="tp2")
                nc.tensor.transpose(tp[:], acc_all[:, h, SPLIT:NCPAD, :],
                                    ident[:])
                nc.vector.tensor_copy(accT2[:, h, :], tp[:])

            hsegT = smp.tile([128, HC, SLOTS], F32R, tag="hsegT",
                             name="hsegT")
            for h in range(HC):
                hs = ps2p.tile([128, SLOTS], F32, tag="ps2", name="hs")
                _mm(nc, hs[:], accT1[:, h, :], cf1_sb[:],
                    start=True, stop=False)
                _mm(nc, hs[:], accT2[:, h, :], cf2_sb[:],
                    start=False, stop=True)
                nc.vector.tensor_copy(hsegT[:, h, :], hs[:])

            # ---- encT[c, s] = W3.T @ hsegT + b3 x counts ----
            encT_sb = smp.tile([128, CC, SLOTS], F32R, tag="encT",
                               name="encT")
            for c in range(CC):
                ep = ps1p.tile([128, SLOTS], F32, tag="ps1", name="ep")
                nc.tensor.matmul(ep[:], b3row[:, c * 128:(c + 1) * 128],
                                 cntrow[:], start=True, stop=False,
                                 skip_group_check=True)
                for k in range(HC):
                    _mm(nc, ep[:], w3_sb[k][:, c * 128:(c + 1) * 128],
                        hsegT[:, k, :], start=False, stop=(k == HC - 1))
                nc.vector.tensor_copy(encT_sb[:, c, :], ep[:])

            # ---- predictor MLP on this core's own <=SLOTS segment rows ----
            q1_sb = smp.tile([128, HC, SLOTS], F32R, tag="q1", name="q1")
            for h in range(HC):
                pp1 = ps2p.tile([128, SLOTS], F32, tag="ps2", name="pp1")
                for c in range(CC):
                    _mm(nc, pp1[:], p1_sb[c][h], encT_sb[:, c, :],
                        start=(c == 0), stop=(c == CC - 1))
                nc.scalar.activation(q1_sb[:, h, :], pp1[:],
                                     mybir.ActivationFunctionType.Tanh,
                                     bias=pb1_sb[:, h:h + 1])
            q2_sb = smp.tile([128, HC, SLOTS], F32R, tag="q2", name="q2")
            for h in range(HC):
                pp2 = ps1p.tile([128, SLOTS], F32, tag="ps1", name="pp2")
                for k in range(HC):
                    _mm(nc, pp2[:], p2_sb[k][h], q1_sb[:, k, :],
                        start=(k == 0), stop=(k == HC - 1))
                nc.scalar.activation(q2_sb[:, h, :], pp2[:],
                                     mybir.ActivationFunctionType.Tanh,
                                     bias=pb2_sb[:, h:h + 1])

            # final: pred[slot, o] = q2.T @ P3 + pb3
            ppo = ps2p.tile([SLOTS, O], F32, tag="ps2", name="ppo")
            nc.tensor.matmul(ppo[:], ones1[:], pb3row[:],
                             start=True, stop=False, skip_group_check=True)
            for k in range(HC):
                _mm(nc, ppo[:], q2_sb[:, k, :], p3_sb[k],
                    start=False, stop=(k == HC - 1))
            pred_sb = smp.tile([SLOTS, O], F32, tag="pred", name="predsb")
            nc.vector.tensor_copy(pred_sb[:], ppo[:])
            nc.sync.dma_start(out_d.ap(), pred_sb[:])

    nc.compile()
    return nc


def kernel(words, seg_ids, W1, b1, W2, b2, W3, b3,
           P1, pb1, P2, pb2, P3, pb3, batch_size, alpha_iter, **_):
    words = np.asarray(words, dtype=np.float32)
    seg_ids = np.asarray(seg_ids).astype(np.int64)
    assert words.shape == (T, E), words.shape
    bs, ai = int(batch_size), int(alpha_iter)
    assert bs * ai == S

    # --- host-side index prep: cut the sorted token axis at segment
    # boundaries so each core owns whole segments ---
    counts = np.bincount(seg_ids, minlength=S)[:S]
    starts = np.concatenate([[0], np.cumsum(counts)])   # [S+1]
    cuts = [0]
    for c in range(1, N_CORES):
        tgt = c * T // N_CORES
        j = int(np.searchsorted(starts, tgt, side="left"))
        if j > 0 and tgt - starts[j - 1] < starts[j] - tgt:
            j -= 1
        cuts.append(int(starts[j]))
    cuts.append(T)
    lens = np.diff(cuts)
    t_sh = int(np.ceil(lens.max() / 128) * 128)

    seg_lo = [0] * N_CORES
    for c in range(N_CORES - 1, 0, -1):
        if lens[c] > 0:
            seg_lo[c] = int(seg_ids[cuts[c]])
        else:
            seg_lo[c] = S if c == N_CORES - 1 else seg_lo[c + 1]
    seg_hi = seg_lo[1:] + [S]
    slots_needed = max(seg_hi[c] - seg_lo[c] for c in range(N_CORES))
    SLOTS = min(128, max(MIN_SLOTS, ((slots_needed + 31) // 32) * 32))
    assert slots_needed <= SLOTS, (seg_lo, seg_hi)

    xt = np.ascontiguousarray(words.T.astype(np.float16))    # [E, T] fp16

    key = ("nc", t_sh, SLOTS)
    if key not in _CACHE:
        _CACHE[key] = _build_nc(t_sh, SLOTS)
    nc = _CACHE[key]

    NC = (t_sh + TOK - 1) // TOK
    iota = np.broadcast_to(np.arange(TOK, dtype=np.float16),
                           (128, TOK)).copy()

    common = {
        "w1": np.ascontiguousarray(W1, dtype=np.float16),
        "w2": np.ascontiguousarray(W2, dtype=np.float16),
        "w3": np.ascontiguousarray(W3, dtype=np.float32),
        "b1": np.ascontiguousarray(b1, dtype=np.float32).reshape(H // 128, 128),
        "b2": np.ascontiguousarray(b2, dtype=np.float32).reshape(H // 128, 128),
        "b3": np.ascontiguousarray(b3, dtype=np.float32).reshape(1, C),
        "p1": np.ascontiguousarray(P1, dtype=np.float32),
        "p2": np.ascontiguousarray(P2, dtype=np.float32),
        "p3": np.ascontiguousarray(P3, dtype=np.float32),
        "pb1": np.ascontiguousarray(pb1, dtype=np.float32).reshape(H // 128, 128),
        "pb2": np.ascontiguousarray(pb2, dtype=np.float32).reshape(H // 128, 128),
        "pb3": np.ascontiguousarray(pb3, dtype=np.float32).reshape(1, O),
        "iota": iota,
    }
    in_maps = []
    for c in range(N_CORES):
        lo, hi = cuts[c], cuts[c + 1]
        n = hi - lo
        xt_c = np.zeros((E, t_sh), dtype=np.float16)
        xt_c[:, :n] = xt[:, lo:hi]
        sl = seg_ids[lo:hi] - seg_lo[c]          # local slot per token
        nseg = seg_hi[c] - seg_lo[c]
        assert n == 0 or (sl.min() >= 0 and sl.max() < SLOTS)

        bm1 = np.zeros(NCPAD, dtype=np.float32)
        bm2 = np.zeros(NCPAD, dtype=np.float32)
        M1 = np.zeros((NCPAD, SLOTS), dtype=np.float32)
        M2 = np.zeros((NCPAD, SLOTS), dtype=np.float32)
        M3 = np.zeros((NCPAD, SLOTS), dtype=np.float32)
        base = 0
        ci = 0
        while base < t_sh:
            ct = min(TOK, t_sh - base)
            nn = min(max(n - base, 0), ct)       # valid tokens in chunk
            if nn > 0:
                ss = sl[base:base + nn]
                bs_pos = (np.nonzero(np.diff(ss))[0] + 1).tolist()
                assert len(bs_pos) <= 2, (c, ci, len(bs_pos))
                if len(bs_pos) == 0:
                    b1c, b2c = nn, nn
                    M1[ci, ss[0]] = 1.0
                elif len(bs_pos) == 1:
                    b1c, b2c = bs_pos[0], nn
                    M1[ci, ss[0]] = 1.0
                    M2[ci, ss[b1c]] = 1.0
                else:
                    assert nn == ct, "two boundaries + padding in one chunk"
                    b1c, b2c = bs_pos
                    M1[ci, ss[0]] = 1.0
                    M2[ci, ss[b1c]] = 1.0
                    M3[ci, ss[b2c]] = 1.0
                bm1[ci], bm2[ci] = b1c, b2c
            base += ct
            ci += 1

        cnt_c = np.zeros((1, SLOTS), dtype=np.float32)
        cnt_c[0, :nseg] = counts[seg_lo[c]:seg_hi[c]]
        A, B, Cm = M1 - M2, M2 - M3, M3
        X = np.stack([A, B, Cm], axis=1).reshape(3 * NCPAD, SLOTS)
        cf1 = X[0:3 * SPLIT]
        cf2 = X[3 * SPLIT:]
        in_maps.append({
            **common,
            "xt": xt_c,
            "cnt": cnt_c,
            "bm1": np.broadcast_to(bm1, (128, NCPAD)).copy(),
            "bm2": np.broadcast_to(bm2, (128, NCPAD)).copy(),
            "cf1": np.ascontiguousarray(cf1),
            "cf2": np.ascontiguousarray(cf2),
        })

    global _LAST_IN_MAPS
    _LAST_IN_MAPS = in_maps
    res = bass_utils.run_bass_kernel_spmd(nc, in_maps,
                                          core_ids=list(range(N_CORES)))
    pred = np.zeros((S, O), dtype=np.float32)
    for c in range(N_CORES):
        nseg = seg_hi[c] - seg_lo[c]
        if nseg > 0:
            pred[seg_lo[c]:seg_hi[c]] = res.results[c]["pred"][:nseg]
    return pred.reshape(bs, ai, O).astype(np.float32)


_LAST_IN_MAPS = None


# revision 10
# speedup vs baseline: 1.0722x; 1.0722x over previous
"""DeepSetPred Trainium2 kernel: 3-layer token encoder MLP + segment-sum +
predictor MLP on 8 NeuronCores, ZERO collectives.

Key structural wins over the naive mapping:

1. The third encoder layer is linear, so it commutes with the segment-sum:
       enc = segsum(h2 @ W3 + b3) = segsum(h2) @ W3 + counts * b3
   The per-token L3 matmul (25% of encoder FLOPs) and the one-hot
   segment-matmul collapse into one tiny [S,H]x[H,C] matmul after pooling.

2. segsum(h2) is computed without materializing anything token-major:
   - The host lays each core's (sorted) tokens out so every 1024-token
     chunk contains at most ONE segment transition, inserting a few zero
     pad tokens (measured <= ~110 per core, usually 0) when two
     boundaries would share a chunk.
   - The L2 tanh on the ScalarEngine emits accum_out = full-chunk sum per
     H-partition (free side output).
   - One DVE scalar_tensor_tensor per (h-tile, chunk) computes the head
     sum  sum_{t < b} h2[:, t]  via the fused (iota is_lt b) mult h2 ->
     accum_out form, with b per chunk as DATA, so the instruction stream
     is identical across all 8 cores -> one NEFF.
   - Pad tokens all produce the identical vector v = tanh(W2'tanh(b1)+b2);
     v is captured on-device from a guaranteed pad column and subtracted
     exactly via one extra row of the assembly matmul.
   - hsegT[h, s] = sum_c head_c*XA[c,s] + full_c*XB[c,s] - npad*v, with
     the coefficient stack X as per-core input data, evaluated by tiny PE
     matmuls after transposing the [128, nchunk, 2] accumulator (round 1
     of the transpose runs mid-loop, overlapped with the token chunks).

3. Everything stays fp16 on the matmul path (fp8 fails here: the ragged
   pooling amplifies per-token quantization noise by sqrt(count) ~ 32x).

Sharding: host cuts the sorted token axis at segment boundaries so every
segment belongs to exactly one core (no collectives); each core runs the
predictor on its own <=SLOTS segments and writes its slice of the output.
"""

import numpy as np

import concourse.mybir as mybir
import concourse.tile as tile
from concourse import bacc
from concourse import bass_utils
from concourse.masks import make_identity

# Problem shapes (hardcoded per contract).
T, E, H, C, O = 131072, 256, 512, 256, 32
S = 128            # num segments
N_CORES = 8
TOK = 1024         # tokens per chunk
NCPAD = 32         # padded chunk-count (accumulator columns; last = v)
SPLIT = 12         # accumulator columns assembled early (round 1)
VCOL = NCPAD - 1   # accumulator column holding the pad vector v
MIN_SLOTS = 32     # segments-per-core capacity
F32 = mybir.dt.float32
F32R = mybir.dt.float32r
F16 = mybir.dt.float16

_CACHE = {}


def _mm(nc, out, lhsT, rhs, start, stop, skip=True):
    nc.tensor.matmul(out, lhsT, rhs,
                     start=start, stop=stop, skip_group_check=skip)


def _build_nc(t_sh, SLOTS):
    assert t_sh % 128 == 0
    NC = (t_sh + TOK - 1) // TOK
    assert NC <= VCOL

    nc = bacc.Bacc("TRN2", target_bir_lowering=False, debug=False,
                   num_devices=N_CORES)

    xt_d = nc.dram_tensor("xt", [E, t_sh], F16, kind="ExternalInput")
    w1_d = nc.dram_tensor("w1", [E, H], F16, kind="ExternalInput")
    w2_d = nc.dram_tensor("w2", [H, H], F16, kind="ExternalInput")
    w3_d = nc.dram_tensor("w3", [H, C], F32R, kind="ExternalInput")
    b1_d = nc.dram_tensor("b1", [H // 128, 128], F32, kind="ExternalInput")
    b2_d = nc.dram_tensor("b2", [H // 128, 128], F32, kind="ExternalInput")
    b3_d = nc.dram_tensor("b3", [1, C], F32, kind="ExternalInput")
    p1_d = nc.dram_tensor("p1", [C, H], F32R, kind="ExternalInput")
    p2_d = nc.dram_tensor("p2", [H, H], F32R, kind="ExternalInput")
    p3_d = nc.dram_tensor("p3", [H, O], F32R, kind="ExternalInput")
    pb1_d = nc.dram_tensor("pb1", [H // 128, 128], F32, kind="ExternalInput")
    pb2_d = nc.dram_tensor("pb2", [H // 128, 128], F32, kind="ExternalInput")
    pb3_d = nc.dram_tensor("pb3", [1, O], F32, kind="ExternalInput")
    cnt_d = nc.dram_tensor("cnt", [1, SLOTS], F32, kind="ExternalInput")
    iota_d = nc.dram_tensor("iota", [128, TOK], F16, kind="ExternalInput")
    bm1_d = nc.dram_tensor("bm1", [128, NCPAD], F32, kind="ExternalInput")
    cf1_d = nc.dram_tensor("cf1", [2 * SPLIT, SLOTS], F32,
                           kind="ExternalInput")
    cf2_d = nc.dram_tensor("cf2", [2 * (NCPAD - SPLIT), SLOTS], F32,
                           kind="ExternalInput")
    out_d = nc.dram_tensor("pred", [SLOTS, O], F32, kind="ExternalOutput")

    EC = E // 128   # 2
    HC = H // 128   # 4
    CC = C // 128   # 2
    LT = mybir.AluOpType.is_lt
    MUL = mybir.AluOpType.mult

    with tile.TileContext(nc) as tc:
        with tc.tile_pool(name="wts", bufs=1) as wp, \
             tc.tile_pool(name="xt", bufs=3) as xtp, \
             tc.tile_pool(name="h1", bufs=2) as h1p, \
             tc.tile_pool(name="h2", bufs=2) as h2p, \
             tc.tile_pool(name="scr", bufs=3) as scp, \
             tc.tile_pool(name="small", bufs=1) as smp, \
             tc.tile_pool(name="ps1", bufs=2, space="PSUM") as ps1p, \
             tc.tile_pool(name="ps2", bufs=2, space="PSUM") as ps2p:

            # warm the ACT tanh table before the scalar queue fills with DMAs
            warm_sb = smp.tile([1, 1], F32, tag="warm", name="warm")
            nc.gpsimd.memset(warm_sb[:], 0.0)
            warm_o = smp.tile([1, 1], F32, tag="warmo", name="warmo")
            nc.scalar.activation(warm_o[:], warm_sb[:],
                                 mybir.ActivationFunctionType.Tanh)

            # ---- resident weights (loop-critical first) ----
            w1_t = wp.tile([128, EC, HC, 128], F16, tag="w1", name="w1t")
            nc.scalar.dma_start(
                w1_t[:], w1_d.ap().rearrange("(e p) (h q) -> p e h q",
                                             p=128, q=128))
            w1_sb = [[w1_t[:, e, h, :] for h in range(HC)] for e in range(EC)]
            # b1/b2 ride the scalar ring: the tanh needs them early and they
            # must not queue behind the epilogue-weight DMAs on gpsimd
            b1_sb = smp.tile([128, HC], F32, tag="b1", name="b1")
            nc.scalar.dma_start(b1_sb[:], b1_d.ap().rearrange("h p -> p h"))
            b2_sb = smp.tile([128, HC], F32, tag="b2", name="b2")
            nc.scalar.dma_start(b2_sb[:], b2_d.ap().rearrange("h p -> p h"))
            w2_t = wp.tile([128, HC, HC, 128], F16, tag="w2", name="w2t")
            nc.scalar.dma_start(
                w2_t[:], w2_d.ap().rearrange("(k p) (h q) -> p k h q",
                                             p=128, q=128))
            w2_sb = [[w2_t[:, k, h, :] for h in range(HC)] for k in range(HC)]
            iota_sb = smp.tile([128, TOK], F16, tag="iota", name="iota")
            nc.gpsimd.dma_start(iota_sb[:], iota_d.ap())
            bm1_sb = smp.tile([128, NCPAD], F32, tag="bm1", name="bm1")
            nc.gpsimd.dma_start(bm1_sb[:], bm1_d.ap())
            # epilogue-only tensors
            w3_t = wp.tile([128, HC, C], F32R, tag="w3", name="w3t")
            nc.gpsimd.dma_start(
                w3_t[:], w3_d.ap().rearrange("(k p) c -> p k c", p=128))
            w3_sb = [w3_t[:, k, :] for k in range(HC)]
            pb1_sb = smp.tile([128, HC], F32, tag="pb1", name="pb1")
            nc.gpsimd.dma_start(pb1_sb[:], pb1_d.ap().rearrange("h p -> p h"))
            pb2_sb = smp.tile([128, HC], F32, tag="pb2", name="pb2")
            nc.gpsimd.dma_start(pb2_sb[:], pb2_d.ap().rearrange("h p -> p h"))
            b3row = smp.tile([1, C], F32, tag="b3row", name="b3row")
            nc.gpsimd.dma_start(b3row[:], b3_d.ap())
            pb3row = smp.tile([1, O], F32, tag="pb3row", name="pb3row")
            nc.gpsimd.dma_start(pb3row[:], pb3_d.ap())
            cntrow = smp.tile([1, SLOTS], F32, tag="cntrow", name="cntrow")
            nc.gpsimd.dma_start(cntrow[:], cnt_d.ap())
            ones1 = smp.tile([1, SLOTS], F32, tag="ones1", name="ones1")
            nc.gpsimd.memset(ones1[:], 1.0)
            ident = smp.tile([128, 128], F32, tag="ident", name="ident")
            make_identity(nc, ident[:])
            cf1_sb = smp.tile([2 * SPLIT, SLOTS], F32, tag="cf1", name="cf1")
            nc.gpsimd.dma_start(cf1_sb[:], cf1_d.ap())
            cf2_sb = smp.tile([2 * (NCPAD - SPLIT), SLOTS], F32, tag="cf2",
                              name="cf2")
            nc.gpsimd.dma_start(cf2_sb[:], cf2_d.ap())

            p1_t = wp.tile([128, CC, HC, 128], F32R, tag="p1", name="p1t")
            nc.gpsimd.dma_start(
                p1_t[:], p1_d.ap().rearrange("(c p) (h q) -> p c h q",
                                             p=128, q=128))
            p1_sb = [[p1_t[:, c, h, :] for h in range(HC)] for c in range(CC)]
            p2_t = wp.tile([128, HC, HC, 128], F32R, tag="p2", name="p2t")
            nc.gpsimd.dma_start(
                p2_t[:], p2_d.ap().rearrange("(k p) (h q) -> p k h q",
                                             p=128, q=128))
            p2_sb = [[p2_t[:, k, h, :] for h in range(HC)] for k in range(HC)]
            p3_t = wp.tile([128, HC, O], F32R, tag="p3", name="p3t")
            nc.gpsimd.dma_start(
                p3_t[:], p3_d.ap().rearrange("(k p) o -> p k o", p=128))
            p3_sb = [p3_t[:, k, :] for k in range(HC)]

            # ---- per-(h,chunk) pooling accumulators (fp32), laid out
            # [h, chunk, plane]: plane 0 = head sum, plane 1 = full sum;
            # column VCOL plane 0 holds the on-device pad vector v ----
            acc_all = smp.tile([128, HC, NCPAD, 2], F32, tag="acc",
                               name="acc")
            nc.gpsimd.memset(acc_all[:], 0.0)

            # ---- main token loop, interleaved at h-block granularity:
            # L1(i+1, h) then L2(i, h) so the scalar/vector queues never
            # sit behind a full chunk of the other layer ----
            chunks = []
            base = 0
            while base < t_sh:
                ct = min(TOK, t_sh - base)
                chunks.append((base, ct))
                base += ct
            assert len(chunks) == NC

            def halves(ct):
                if ct <= 512:
                    return [(0, ct)]
                return [(0, 512), (512, ct - 512)]

            def load_chunk(ci, base, ct):
                xt_t = xtp.tile([128, EC, ct], F16, tag="xt", name="xt",
                                padded_shape=[128, EC, TOK])
                nc.sync.dma_start(
                    xt_t[:],
                    xt_d.ap()[:, base:base + ct]
                        .rearrange("(e p) t -> p e t", p=128))
                h1_t = h1p.tile([128, HC, ct], F16, tag="h1", name="h1",
                                padded_shape=[128, HC, TOK])
                h2_t = h2p.tile([128, HC, ct], F16, tag="h2", name="h2",
                                padded_shape=[128, HC, TOK])
                return (ci, base, ct, xt_t, h1_t, h2_t)

            def l1_h(st, h):
                ci, base, ct, xt_t, h1_t, _ = st
                ps1 = ps1p.tile([128, ct], F32, tag="ps1", name="ps1",
                                padded_shape=[128, TOK])
                for (hb, hl) in halves(ct):
                    for e in range(EC):
                        _mm(nc, ps1[:, hb:hb + hl], w1_sb[e][h],
                            xt_t[:, e, hb:hb + hl],
                            start=(e == 0), stop=(e == EC - 1))
                nc.scalar.activation(h1_t[:, h, :], ps1[:],
                                     mybir.ActivationFunctionType.Tanh,
                                     bias=b1_sb[:, h:h + 1])

            def l2_h(st, h):
                ci, base, ct, _, h1_t, h2_t = st
                ps2 = ps2p.tile([128, ct], F32, tag="ps2", name="ps2",
                                padded_shape=[128, TOK])
                for (hb, hl) in halves(ct):
                    for k in range(HC):
                        _mm(nc, ps2[:, hb:hb + hl], w2_sb[k][h],
                            h1_t[:, k, hb:hb + hl],
                            start=(k == 0), stop=(k == HC - 1))
                nc.scalar.activation(h2_t[:, h, :], ps2[:],
                                     mybir.ActivationFunctionType.Tanh,
                                     bias=b2_sb[:, h:h + 1],
                                     accum_out=acc_all[:, h, ci, 1:2])
                # head sum before the (<=1) intra-chunk boundary:
                # (iota < b) * h2 summed along tokens, b is per-core data
                sc = scp.tile([128, ct], F16, tag="scr", name="scr",
                              padded_shape=[128, TOK])
                nc.vector.scalar_tensor_tensor(
                    sc[:], iota_sb[:, :ct], bm1_sb[:, ci:ci + 1],
                    h2_t[:, h, :], LT, MUL,
                    accum_out=acc_all[:, h, ci, 0:1])

            # round-1 accumulator assembly: transpose chunk columns
            # [0:SPLIT] once those chunks are done; overlaps the rest
            accT1 = smp.tile([2 * SPLIT, HC, 128], F32, tag="accT1",
                             name="accT1")

            def round1():
                for h in range(HC):
                    tp = ps1p.tile([2 * SPLIT, 128], F32, tag="ps1",
                                   name="tp1")
                    nc.tensor.transpose(tp[:], acc_all[:, h, 0:SPLIT, :],
                                        ident[:])
                    nc.vector.tensor_copy(accT1[:, h, :], tp[:])

            prev = None
            done = 0
            for ci, (base, ct) in enumerate(chunks):
                cur = load_chunk(ci, base, ct)
                for h in range(HC):
                    l1_h(cur, h)
                    if prev is not None:
                        l2_h(prev, h)
                if prev is not None:
                    done += 1
                    if done == SPLIT:
                        round1()
                prev = cur
            for h in range(HC):
                l2_h(prev, h)
            done += 1
            if done == SPLIT:
                round1()

            # capture the pad vector v from the guaranteed trailing pad
            # column of the last chunk (all pad tokens produce identical
            # h2, so this correction is exact)
            ct_last = chunks[-1][1]
            h2_last = prev[5]
            for h in range(HC):
                nc.vector.tensor_copy(acc_all[:, h, VCOL, 0:1],
                                      h2_last[:, h, ct_last - 1:ct_last])

            # ---- assemble hsegT: round 2 over the remaining chunk
            # columns (incl. the v column), then 2 tiny matmuls per h ----
            accT2 = smp.tile([2 * (NCPAD - SPLIT), HC, 128], F32,
                             tag="accT2", name="accT2")
            for h in range(HC):
                tp = ps1p.tile([2 * (NCPAD - SPLIT), 128], F32, tag="ps1",
                               name="tp2")
                nc.tensor.transpose(tp[:], acc_all[:, h, SPLIT:NCPAD, :],
                                    ident[:])
                nc.vector.tensor_copy(accT2[:, h, :], tp[:])

            hsegT = smp.tile([128, HC, SLOTS], F32R, tag="hsegT",
                             name="hsegT")
            for h in range(HC):
                hs = ps2p.tile([128, SLOTS], F32, tag="ps2", name="hs")
                _mm(nc, hs[:], accT1[:, h, :], cf1_sb[:],
                    start=True, stop=False)
                _mm(nc, hs[:], accT2[:, h, :], cf2_sb[:],
                    start=False, stop=True)
                nc.vector.tensor_copy(hsegT[:, h, :], hs[:])

            # ---- encT[c, s] = W3.T @ hsegT + b3 x counts ----
            encT_sb = smp.tile([128, CC, SLOTS], F32R, tag="encT",
                               name="encT")
            for c in range(CC):
                ep = ps1p.tile([128, SLOTS], F32, tag="ps1", name="ep")
                nc.tensor.matmul(ep[:], b3row[:, c * 128:(c + 1) * 128],
                                 cntrow[:], start=True, stop=False,
                                 skip_group_check=True)
                for k in range(HC):
                    _mm(nc, ep[:], w3_sb[k][:, c * 128:(c + 1) * 128],
                        hsegT[:, k, :], start=False, stop=(k == HC - 1))
                nc.vector.tensor_copy(encT_sb[:, c, :], ep[:])

            # ---- predictor MLP on this core's own <=SLOTS segment rows ----
            q1_sb = smp.tile([128, HC, SLOTS], F32R, tag="q1", name="q1")
            for h in range(HC):
                pp1 = ps2p.tile([128, SLOTS], F32, tag="ps2", name="pp1")
                for c in range(CC):
                    _mm(nc, pp1[:], p1_sb[c][h], encT_sb[:, c, :],
                        start=(c == 0), stop=(c == CC - 1))
                nc.scalar.activation(q1_sb[:, h, :], pp1[:],
                                     mybir.ActivationFunctionType.Tanh,
                                     bias=pb1_sb[:, h:h + 1])
            q2_sb = smp.tile([128, HC, SLOTS], F32R, tag="q2", name="q2")
            for h in range(HC):
                pp2 = ps1p.tile([128, SLOTS], F32, tag="ps1", name="pp2")
                for k in range(HC):
                    _mm(nc, pp2[:], p2_sb[k][h], q1_sb[:, k, :],
                        start=(k == 0), stop=(k == HC - 1))
                nc.scalar.activation(q2_sb[:, h, :], pp2[:],
                                     mybir.ActivationFunctionType.Tanh,
                                     bias=pb2_sb[:, h:h + 1])

            # final: pred[slot, o] = q2.T @ P3 + pb3
            ppo = ps2p.tile([SLOTS, O], F32, tag="ps2", name="ppo")
            nc.tensor.matmul(ppo[:], ones1[:], pb3row[:],
                             start=True, stop=False, skip_group_check=True)
            for k in range(HC):
                _mm(nc, ppo[:], q2_sb[:, k, :], p3_sb[k],
                    start=False, stop=(k == HC - 1))
            pred_sb = smp.tile([SLOTS, O], F32, tag="pred", name="predsb")
            nc.vector.tensor_copy(pred_sb[:], ppo[:])
            nc.sync.dma_start(out_d.ap(), pred_sb[:])

    nc.compile()
    return nc


def _layout_core(sl):
    """Place one core's tokens (already sorted by local slot `sl`) with pad
    insertion so no 1024-token chunk contains two segment transitions.
    Returns src index array (-1 = pad) into the core's local token order."""
    n = len(sl)
    seg_starts = [0] + (np.nonzero(np.diff(sl))[0] + 1).tolist() + [n]
    pos = 0
    out = []
    trans = {}
    for i in range(len(seg_starts) - 1):
        lo, hi = seg_starts[i], seg_starts[i + 1]
        if i > 0:
            ch = pos // TOK
            if pos % TOK != 0 and trans.get(ch, 0) >= 1:
                pad = TOK - (pos % TOK)
                out.extend([-1] * pad)
                pos += pad
            if pos % TOK != 0:
                ch = pos // TOK
                trans[ch] = trans.get(ch, 0) + 1
        out.extend(range(lo, hi))
        pos += hi - lo
    return np.array(out, dtype=np.int64)


def kernel(words, seg_ids, W1, b1, W2, b2, W3, b3,
           P1, pb1, P2, pb2, P3, pb3, batch_size, alpha_iter, **_):
    words = np.asarray(words, dtype=np.float32)
    seg_ids = np.asarray(seg_ids).astype(np.int64)
    assert words.shape == (T, E), words.shape
    bs, ai = int(batch_size), int(alpha_iter)
    assert bs * ai == S

    # --- host-side index prep: cut the sorted token axis at segment
    # boundaries so each core owns whole segments ---
    counts = np.bincount(seg_ids, minlength=S)[:S]
    starts = np.concatenate([[0], np.cumsum(counts)])   # [S+1]
    cuts = [0]
    for c in range(1, N_CORES):
        tgt = c * T // N_CORES
        j = int(np.searchsorted(starts, tgt, side="left"))
        if j > 0 and tgt - starts[j - 1] < starts[j] - tgt:
            j -= 1
        cuts.append(int(starts[j]))
    cuts.append(T)
    lens = np.diff(cuts)

    seg_lo = [0] * N_CORES
    for c in range(N_CORES - 1, 0, -1):
        if lens[c] > 0:
            seg_lo[c] = int(seg_ids[cuts[c]])
        else:
            seg_lo[c] = S if c == N_CORES - 1 else seg_lo[c + 1]
    seg_hi = seg_lo[1:] + [S]
    slots_needed = max(seg_hi[c] - seg_lo[c] for c in range(N_CORES))
    SLOTS = min(128, max(MIN_SLOTS, ((slots_needed + 31) // 32) * 32))
    assert slots_needed <= SLOTS, (seg_lo, seg_hi)

    # per-core padded layouts; t_sh must leave >=1 trailing pad column so
    # the kernel can capture the pad vector v from the last chunk
    layouts = []
    for c in range(N_CORES):
        sl = seg_ids[cuts[c]:cuts[c + 1]] - seg_lo[c]
        layouts.append(_layout_core(sl))
    maxlen = max(len(lm) for lm in layouts)
    t_sh = int(np.ceil((maxlen + 1) / 128) * 128)

    xt = np.ascontiguousarray(words.T.astype(np.float16))    # [E, T] fp16

    key = ("nc", t_sh, SLOTS)
    if key not in _CACHE:
        _CACHE[key] = _build_nc(t_sh, SLOTS)
    nc = _CACHE[key]

    NC = (t_sh + TOK - 1) // TOK
    iota = np.broadcast_to(np.arange(TOK, dtype=np.float16),
                           (128, TOK)).copy()

    common = {
        "w1": np.ascontiguousarray(W1, dtype=np.float16),
        "w2": np.ascontiguousarray(W2, dtype=np.float16),
        "w3": np.ascontiguousarray(W3, dtype=np.float32),
        "b1": np.ascontiguousarray(b1, dtype=np.float32).reshape(H // 128, 128),
        "b2": np.ascontiguousarray(b2, dtype=np.float32).reshape(H // 128, 128),
        "b3": np.ascontiguousarray(b3, dtype=np.float32).reshape(1, C),
        "p1": np.ascontiguousarray(P1, dtype=np.float32),
        "p2": np.ascontiguousarray(P2, dtype=np.float32),
        "p3": np.ascontiguousarray(P3, dtype=np.float32),
        "pb1": np.ascontiguousarray(pb1, dtype=np.float32).reshape(H // 128, 128),
        "pb2": np.ascontiguousarray(pb2, dtype=np.float32).reshape(H // 128, 128),
        "pb3": np.ascontiguousarray(pb3, dtype=np.float32).reshape(1, O),
        "iota": iota,
    }
    in_maps = []
    for c in range(N_CORES):
        lo, hi = cuts[c], cuts[c + 1]
        sl = seg_ids[lo:hi] - seg_lo[c]          # local slot per token
        nseg = seg_hi[c] - seg_lo[c]
        lm = layouts[c]
        n = len(lm)
        assert n < t_sh                          # trailing pad guaranteed
        src = np.full(t_sh, -1, dtype=np.int64)
        src[:n] = lm
        valid = src >= 0
        xt_c = np.zeros((E, t_sh), dtype=np.float16)
        xt_c[:, valid] = xt[:, lo + src[valid]]
        slot_of = np.full(t_sh, -1, dtype=np.int64)
        slot_of[valid] = sl[src[valid]]

        bm1 = np.zeros(NCPAD, dtype=np.float32)
        XA = np.zeros((NCPAD, SLOTS), dtype=np.float32)
        XB = np.zeros((NCPAD, SLOTS), dtype=np.float32)
        npad_of_slot = np.zeros(SLOTS, dtype=np.float32)
        base = 0
        for ci in range(NC):
            ct = min(TOK, t_sh - base)
            ss = slot_of[base:base + ct]
            nz = np.nonzero((ss[1:] != ss[:-1]) & (ss[1:] >= 0)
                            & (ss[:-1] >= 0))[0]
            assert len(nz) <= 1, (c, ci, len(nz))
            vidx = np.nonzero(ss >= 0)[0]
            if len(vidx) == 0:
                base += ct
                continue
            if len(nz) == 1:                     # [A | B (+pads)] chunk
                b = int(nz[0]) + 1
                segA, segB = int(ss[0]), int(ss[b])
                padB = int((ss[b:] == -1).sum())
                XA[ci, segA] += 1.0
                XA[ci, segB] -= 1.0              # B gets full - head
                XB[ci, segB] += 1.0
                npad_of_slot[segB] += padB
            else:                                # [A (+pads)] chunk
                b = int(vidx[-1]) + 1
                XA[ci, int(ss[0])] += 1.0
            bm1[ci] = b
            base += ct

        cnt_c = np.zeros((1, SLOTS), dtype=np.float32)
        cnt_c[0, :nseg] = counts[seg_lo[c]:seg_hi[c]]
        # coefficient stack rows (chunk-major, plane-minor); the v column
        # (VCOL, plane 0) subtracts npad * v for pad-polluted B parts
        X = np.stack([XA, XB], axis=1).reshape(2 * NCPAD, SLOTS)
        X[2 * VCOL + 0] = -npad_of_slot
        in_maps.append({
            **common,
            "xt": xt_c,
            "cnt": cnt_c,
            "bm1": np.broadcast_to(bm1, (128, NCPAD)).copy(),
            "cf1": np.ascontiguousarray(X[0:2 * SPLIT]),
            "cf2": np.ascontiguousarray(X[2 * SPLIT:]),
        })

    global _LAST_IN_MAPS
    _LAST_IN_MAPS = in_maps
    res = bass_utils.run_bass_kernel_spmd(nc, in_maps,
                                          core_ids=list(range(N_CORES)))
    pred = np.zeros((S, O), dtype=np.float32)
    for c in range(N_CORES):
        nseg = seg_hi[c] - seg_lo[c]
        if nseg > 0:
            pred[seg_lo[c]:seg_hi[c]] = res.results[c]["pred"][:nseg]
    return pred.reshape(bs, ai, O).astype(np.float32)


_LAST_IN_MAPS = None
